# revision 10
# baseline (speedup 1.0000x reference)
"""NeighborDiscriminator kernel for 8x Trainium2 NeuronCores.

Math (reference): augmented-L2 kNN search, k=10, over n=100000 database rows,
B=1024 queries, d=512, followed by max over the k neighbors of
act_i = w_i - ||x_i - q||.

Selection key per (query q, candidate i):
    d2aug = ||q||^2 - 2 q.x_i + ||x_i||^2 + (max(w) - w_i)
Per-query-constant terms don't change the per-query ordering, so the device
ranks by  A = 2 q.x_i + (w_i - ||x_i||^2 - OFF)  (descending A == ascending
d2aug; OFF is a global constant that centers the aug term into fp8 range).

Distribution: X / w sharded over 8 cores along n (12500 rows each), queries
replicated.

Device pipeline per core (all-fp8 DoubleRow matmuls, ~0.5 PE cycles/row):
  - scores A = [128q x 500c] accumulate in PSUM: two fp8e4 DoubleRow matmuls
    (256-deep contraction each) + one DoubleRow rank-1 that adds the aug row
    as coarse+residual fp8 pair (quantization error ~0.06 abs, ~5x below
    typical rank gaps).
  - ACT evicts PSUM -> SBUF as fp16 into a per-query [128, 12504] row.
  - DVE halves that row 3x with tensor_max (2-byte dtype -> 2x mode), giving
    1563 window-maxima (window = 8 candidates with stride 1563), then one
    Max + MaxIndex finds the top-8 windows per query.
Host expands each winning window (8 candidates each, 64/core, 512/query) and
re-ranks exactly: fp32 distance cut to 32, fp64 exact top-k, then
max_k(w - dist).  A true top-10 member is missed only if >=8 distinct
same-core candidates outscore it under ~3-sigma fp8 noise (P ~ 1e-6).
"""

import os

import numpy as np
import ml_dtypes

import concourse.bacc as bacc
import concourse.mybir as mybir
from concourse.tile import TileContext
from concourse.bass_utils import run_bass_kernel_spmd

# matmul flavor: dr8 = fp8 DoubleRow, p8 = plain fp8, bf16 = plain bf16
MODE = os.environ.get("KMODE", "dr8")

B = 1024            # queries
N_TOTAL = 100000    # database rows
D = 512             # feature dim
M = 8               # cores
NS = N_TOTAL // M   # 12500 rows per core
CT = 500            # candidate tile width (PSUM bank = 512 fp32)
KC = D // 128       # 4 contraction chunks (2 DoubleRow pairs)
TOP = 8             # top-8 windows per query per core (DVE max8)
WEXP = 8            # candidates per window (3 halvings)

F8 = ml_dtypes.bfloat16 if MODE == "bf16" else ml_dtypes.float8_e4m3

_cached_nc = None


def _hw(ns):
    """fp16 score-row width: ns padded to a multiple of WEXP."""
    return ((ns + WEXP - 1) // WEXP) * WEXP


def _build(b=B, ns=NS):
    qt = b // 128
    nt = ns // CT
    hw = _hw(ns)
    win = hw // WEXP

    nc = bacc.Bacc(
        "TRN2",
        target_bir_lowering=False,
        debug=False,
        enable_asserts=False,
        num_devices=M,
    )
    f8 = mybir.dt.bfloat16 if MODE == "bf16" else mybir.dt.float8e4
    f16 = mybir.dt.float16
    DR = mybir.MatmulPerfMode.DoubleRow

    q8 = nc.dram_tensor("q8", [D, b], f8, kind="ExternalInput")
    x8 = nc.dram_tensor("x8", [D, ns], f8, kind="ExternalInput")
    aug8 = nc.dram_tensor("aug8", [1, 2, ns], f8, kind="ExternalInput")
    ones8 = nc.dram_tensor("ones8", [1, 2, 128], f8, kind="ExternalInput")
    idxs = nc.dram_tensor("idxs", [b, TOP], mybir.dt.uint16, kind="ExternalOutput")

    with TileContext(nc) as tc:
        nb = 1 if MODE == "bf16" else 2
        with (
            tc.tile_pool(name="const", bufs=1) as cpool,
            tc.tile_pool(name="h0", bufs=nb) as hpool,
            tc.tile_pool(name="casc", bufs=nb) as cascpool,
            tc.tile_pool(name="out", bufs=1) as opool,
            tc.tile_pool(name="ps", bufs=8, space="PSUM") as pspool,
        ):
            q_tile = cpool.tile([128, KC, b], f8)
            nc.sync.dma_start(out=q_tile, in_=q8.rearrange("(c p) m -> p c m", p=128))
            ones_t = cpool.tile([1, 2, 128], f8)
            nc.sync.dma_start(out=ones_t, in_=ones8[:, :, :])
            aug_t = cpool.tile([1, 2, ns], f8)
            nc.sync.dma_start(out=aug_t, in_=aug8[:, :, :])

            # x resident in SBUF (fp8: ns bytes/partition/chunk), chunked DMA
            # so the first matmuls start before the whole 6.4MB lands.
            x8_r = x8.rearrange("(c p) n -> p c n", p=128)
            n_chunks = min(5, nt)
            tpc = (nt + n_chunks - 1) // n_chunks
            xj = []
            for j in range(0, nt, tpc):
                t0, t1 = j * CT, min((j + tpc) * CT, ns)
                xt = cpool.tile([128, KC, t1 - t0], f8, name=f"x{j}")
                nc.sync.dma_start(out=xt, in_=x8_r[:, :, t0:t1])
                xj.append((t0, xt))

            def xslice(t):
                for t0, xt in reversed(xj):
                    if t * CT >= t0:
                        return xt[:, :, t * CT - t0 : (t + 1) * CT - t0]
                raise AssertionError

            i8 = opool.tile([128, qt * TOP], mybir.dt.uint16)
            m8 = opool.tile([128, qt * TOP], f16)

            NEG = -30000.0
            for q in range(qt):
                h = hpool.tile([128, hw], f16, name="h")
                if hw > ns:
                    nc.vector.memset(h[:, ns:], NEG)
                qs = slice(q * 128, (q + 1) * 128)
                for t in range(nt):
                    ps = pspool.tile([128, CT], mybir.dt.float32)
                    xs = xslice(t)
                    ts = slice(t * CT, (t + 1) * CT)
                    if MODE == "dr8":
                        nc.tensor.matmul(
                            ps, lhsT=q_tile[:, 0:2, qs], rhs=xs[:, 0:2, :],
                            start=True, stop=False, perf_mode=DR,
                        )
                        nc.tensor.matmul(
                            ps, lhsT=q_tile[:, 2:4, qs], rhs=xs[:, 2:4, :],
                            start=False, stop=False, perf_mode=DR,
                        )
                        nc.tensor.matmul(
                            ps, lhsT=ones_t, rhs=aug_t[:, :, ts],
                            start=False, stop=True, perf_mode=DR,
                        )
                    else:
                        for c in range(KC):
                            nc.tensor.matmul(
                                ps, lhsT=q_tile[:, c, qs], rhs=xs[:, c, :],
                                start=(c == 0), stop=False,
                            )
                        nc.tensor.matmul(
                            ps, lhsT=ones_t[:, 0, :], rhs=aug_t[:, 0, ts],
                            start=False, stop=False,
                        )
                        nc.tensor.matmul(
                            ps, lhsT=ones_t[:, 1, :], rhs=aug_t[:, 1, ts],
                            start=False, stop=True,
                        )
                    nc.scalar.copy(h[:, ts], ps)

                c1 = cascpool.tile([128, hw // 2], f16)
                nc.vector.tensor_max(c1, h[:, : hw // 2], h[:, hw // 2 :])
                c2 = cascpool.tile([128, hw // 4], f16)
                nc.vector.tensor_max(c2, c1[:, : hw // 4], c1[:, hw // 4 :])
                c3 = cascpool.tile([128, win], f16)
                nc.vector.tensor_max(c3, c2[:, :win], c2[:, win:])
                o = slice(q * TOP, (q + 1) * TOP)
                nc.vector.max(out=m8[:, o], in_=c3)
                nc.vector.max_index(out=i8[:, o], in_max=m8[:, o], in_values=c3)

            nc.sync.dma_start(
                out=idxs.rearrange("(q p) k -> p q k", p=128),
                in_=i8.rearrange("p (q k) -> p q k", q=qt),
            )
    nc.compile()
    return nc


def _get_nc():
    global _cached_nc
    if _cached_nc is None:
        _cached_nc = _build()
    return _cached_nc


def prepare_in_maps(X_tilde, X, w):
    """fp8 device inputs for each core. Shared with test.py's bench."""
    q8 = np.ascontiguousarray((2.0 * X_tilde).T).astype(F8)  # [D, B]
    x_sq = np.einsum("nd,nd->n", X.astype(np.float64), X.astype(np.float64))
    aug = w[:, 0].astype(np.float64) - x_sq                  # [n]
    off = float(np.mean(aug))
    augc = (aug - off).astype(np.float32)
    coarse = augc.astype(F8)
    resid = (augc - coarse.astype(np.float32)).astype(F8)
    ones = np.ones((1, 2, 128), F8)

    in_maps = []
    for c in range(M):
        sl = slice(c * NS, (c + 1) * NS)
        in_maps.append(
            {
                "q8": q8,
                "x8": np.ascontiguousarray(X[sl].T).astype(F8),
                "aug8": np.stack([coarse[sl], resid[sl]])[None, :, :],
                "ones8": ones,
            }
        )
    return in_maps


def _device_candidates(X_tilde, X, w):
    """Run the SPMD search. Returns candidate global rows [B, M*TOP*WEXP]
    plus a validity mask (window expansion can run past the shard edge)."""
    in_maps = prepare_in_maps(X_tilde, X, w)
    res = run_bass_kernel_spmd(_get_nc(), in_maps, core_ids=list(range(M)))
    win = _hw(NS) // WEXP  # 1563
    idx = np.stack([res.results[c]["idxs"] for c in range(M)], axis=1)  # [B,M,8]
    j = idx.astype(np.int64)[:, :, :, None] + win * np.arange(WEXP)[None, None, None, :]
    valid = j < NS                                           # [B,M,8,WEXP]
    rows = np.arange(M)[None, :, None, None] * NS + np.minimum(j, NS - 1)
    return rows.reshape(B, -1), valid.reshape(B, -1)


def kernel(X_tilde, X, w, k):
    k = int(k)
    assert k <= 32, f"host merge sized for small k, got {k}"
    X_tilde = np.asarray(X_tilde, dtype=np.float32)
    X = np.asarray(X, dtype=np.float32)
    w = np.asarray(w, dtype=np.float32).reshape(N_TOTAL, 1)

    cand, valid = _device_candidates(X_tilde, X, w)          # [B, 512]

    # fp32 cut 512 -> 32 by the exact key, then fp64 exact top-k of those.
    x_sq = np.einsum("nd,nd->n", X, X)                       # [n] fp32
    q_sq = np.einsum("bd,bd->b", X_tilde, X_tilde)           # [B]
    Xc = X[cand]                                             # [B, 512, d]
    qx = np.einsum("bkd,bd->bk", Xc, X_tilde)                # [B, 512]
    d2 = q_sq[:, None] - 2.0 * qx + x_sq[cand]
    key = d2 - w[cand, 0]
    key[~valid] = np.inf
    margin = max(32, 2 * k)
    part = np.argpartition(key, margin, axis=1)[:, :margin]
    c32 = np.take_along_axis(cand, part, axis=1)             # [B, 32]

    Xc64 = X[c32].astype(np.float64)
    diff = Xc64 - X_tilde[:, None, :].astype(np.float64)
    d2e = np.einsum("bkd,bkd->bk", diff, diff)               # exact
    wc = w[c32, 0].astype(np.float64)
    key64 = d2e - wc
    sel = np.argpartition(key64, k, axis=1)[:, :k]           # exact k nearest
    d2k = np.take_along_axis(d2e, sel, axis=1)
    wk = np.take_along_axis(wc, sel, axis=1)
    act = wk - np.sqrt(d2k)                                  # K_COEF = 1.0
    return act.max(axis=1).astype(np.float32)
